# revision 2
# baseline (speedup 1.0000x reference)
# Trainium2 Bass kernel for Mixtral block-sparse top-2 MLP with HQQ 4-bit
# quantized weights (dequant + silu-gated MLP), tensor-parallel over 8
# NeuronCores.
#
#   y = (silu(x @ W1.T) * (x @ W3.T)) @ W2.T
#   W* = (Wq - zero) * scale, groups of 64 along the input dim.
#
# Sharding: F (=14336) split 8 ways. Core c holds rows [c*1792, (c+1)*1792)
# of W1/W3 and the matching columns of W2, computes a partial y.T, and the
# host sums the 8 partials (the "all-reduce" of row-parallel W2).
#
# Device-side math (per core, everything fp16 except PSUM f32 accum):
#   Wsc = Wq * scale         (one broadcast-AP tensor_tensor per slab;
#                             int32 -> fp16, scale broadcast over each
#                             64-wide group)
#   zero terms are folded into a rank-|groups| correction matmul:
#     x @ ((Wq - z)*s).T = x @ (Wq*s).T - Xg @ (z*s).T
#   where Xg[t,g] = sum of x[t, h] over group g. Same trick for W2 with
#   Gg[t,fg] group sums of the gated activations.
#
# Layouts: weights are DMA'd in natural layout (dequant needs the non-group
# index on partitions), transposed on the PE (128x128 tiles via identity
# matmul) into contraction-on-partitions layout for the main matmuls.

import os
import numpy as np

N_CORES = 8
T, H, F = 512, 4096, 14336
FC = F // N_CORES        # 1792 rows of W1/W3 (cols of W2) per core
NF = FC // 128           # 14 f-tiles per core
NH = H // 128            # 32 h-tiles
G = 64                   # quantization group size
NG1 = H // G             # 64 groups along H (W1/W3)
GC2 = FC // G            # 28 groups along F per core (W2)
HHALF = H // 2           # dequant slab width for W1/W3

_built = None


def _build():
    from contextlib import ExitStack
    import concourse.bacc as bacc
    import concourse.bass as bass
    import concourse.tile as tile
    import concourse.mybir as mybir

    dt = mybir.dt
    Alu = mybir.AluOpType
    Act = mybir.ActivationFunctionType

    nc = bacc.Bacc("TRN2", target_bir_lowering=False, debug=False)

    w1q_d = nc.dram_tensor("w1q", [FC, H], dt.int32, kind="ExternalInput")
    w3q_d = nc.dram_tensor("w3q", [FC, H], dt.int32, kind="ExternalInput")
    w2q_d = nc.dram_tensor("w2q", [H, FC], dt.int32, kind="ExternalInput")
    s1_d = nc.dram_tensor("s1", [FC, NG1], dt.float32, kind="ExternalInput")
    z1_d = nc.dram_tensor("z1", [FC, NG1], dt.float32, kind="ExternalInput")
    s3_d = nc.dram_tensor("s3", [FC, NG1], dt.float32, kind="ExternalInput")
    z3_d = nc.dram_tensor("z3", [FC, NG1], dt.float32, kind="ExternalInput")
    s2_d = nc.dram_tensor("s2", [H, GC2], dt.float32, kind="ExternalInput")
    z2_d = nc.dram_tensor("z2", [H, GC2], dt.float32, kind="ExternalInput")
    xT_d = nc.dram_tensor("xT", [H, T], dt.float32, kind="ExternalInput")
    eyeh_d = nc.dram_tensor("eye_hf", [128, 128], dt.float16, kind="ExternalInput")
    eyef_d = nc.dram_tensor("eye_f32", [128, 128], dt.float32, kind="ExternalInput")
    B1_d = nc.dram_tensor("B1", [128, NH * G], dt.float16, kind="ExternalInput")
    B2_d = nc.dram_tensor("B2", [128, NF * GC2], dt.float16, kind="ExternalInput")
    yT_d = nc.dram_tensor("yT", [H, T], dt.float32, kind="ExternalOutput")

    def bcast(ap, n):
        # append a stride-0 dim of size n to an AP (free-dim broadcast)
        return bass.AP(ap.tensor, ap.offset, list(ap.ap) + [[0, n]])

    with tile.TileContext(nc) as tc, ExitStack() as ctx:
        cpool = ctx.enter_context(tc.tile_pool(name="consts", bufs=1))
        pmisc = ctx.enter_context(tc.tile_pool(name="pmisc", bufs=2, space="PSUM"))

        eye_hf = cpool.tile([128, 128], dt.float16, tag="eyeh")
        nc.sync.dma_start(eye_hf[:], eyeh_d.ap())
        eye_f32 = cpool.tile([128, 128], dt.float32, tag="eyef")
        nc.sync.dma_start(eye_f32[:], eyef_d.ap())
        B1 = cpool.tile([128, NH * G], dt.float16, tag="B1")
        nc.sync.dma_start(B1[:], B1_d.ap())
        B2 = cpool.tile([128, NF * GC2], dt.float16, tag="B2")
        nc.sync.dma_start(B2[:], B2_d.ap())

        # scales in "tiled-natural" layout: s1_sb[p, i*NGpt + g] = s1[i*128+p, g]
        def load_sc(dram, ngr, ntile):
            t = cpool.tile([128, ntile * ngr], dt.float32, tag=dram.name + "sb")
            nc.sync.dma_start(
                t[:].rearrange("p (i g) -> p i g", g=ngr),
                dram.ap().rearrange("(i p) g -> p i g", p=128),
            )
            return t

        s1_sb = load_sc(s1_d, NG1, NF)
        s3_sb = load_sc(s3_d, NG1, NF)
        s2_sb = load_sc(s2_d, GC2, NH)

        negZS1T = cpool.tile([G, NF * 128], dt.float16, tag="nzs1T")
        negZS3T = cpool.tile([G, NF * 128], dt.float16, tag="nzs3T")
        negZS2T = cpool.tile([GC2, NH * 128], dt.float16, tag="nzs2T")
        XgT = cpool.tile([NG1, T], dt.float16, tag="XgT")
        GgT = cpool.tile([GC2, T], dt.float16, tag="GgT")
        xT_hf = cpool.tile([128, NH * T], dt.float16, tag="xThf")
        gT_hf = cpool.tile([128, NF * T], dt.float16, tag="gThf")

        # ---- x: load f32, cast to fp16 (scoped staging) ----
        with tc.tile_pool(name="xstg", bufs=1) as xstg:
            xs = xstg.tile([128, NH * T], dt.float32, tag="xs")
            nc.sync.dma_start(
                xs[:].rearrange("p (i t) -> p i t", t=T),
                xT_d.ap().rearrange("(i p) t -> p i t", p=128),
            )
            nc.vector.tensor_copy(xT_hf[:], xs[:])

            # ---- negated zero*scale, transposed (small) ----
            def neg_zs_T(z_dram, s_sb, ngr, ntile, outT):
                zs = xstg.tile([128, ntile * ngr], dt.float32, tag="zstg")
                nc.sync.dma_start(
                    zs[:].rearrange("p (i g) -> p i g", g=ngr),
                    z_dram.ap().rearrange("(i p) g -> p i g", p=128),
                )
                nzs = xstg.tile([128, ntile * ngr], dt.float32, tag="nzstg")
                nc.vector.scalar_tensor_tensor(
                    nzs[:], zs[:], -1.0, s_sb[:], op0=Alu.mult, op1=Alu.mult
                )
                for i in range(ntile):
                    pt = pmisc.tile([ngr, 128], dt.float32, tag="misc")
                    nc.tensor.transpose(
                        pt[:], nzs[:, i * ngr : (i + 1) * ngr], eye_f32[:]
                    )
                    nc.scalar.activation(
                        outT[:, i * 128 : (i + 1) * 128], pt[:], Act.Copy
                    )

            neg_zs_T(z1_d, s1_sb, NG1, NF, negZS1T)
            neg_zs_T(z3_d, s3_sb, NG1, NF, negZS3T)
            neg_zs_T(z2_d, s2_sb, GC2, NH, negZS2T)

        # ---- Xg.T: per-group sums of x via block-ones matmuls ----
        xg_ps = pmisc.tile([NG1, T], dt.float32, tag="misc")
        for c in range(NH):
            nc.tensor.matmul(
                xg_ps[:],
                B1[:, c * G : (c + 1) * G],
                xT_hf[:, c * T : (c + 1) * T],
                start=(c == 0),
                stop=(c == NH - 1),
            )
        nc.scalar.activation(XgT[:], xg_ps[:], Act.Copy)

        # ---- main pools ----
        slab = ctx.enter_context(tc.tile_pool(name="slab", bufs=4))
        wscp = ctx.enter_context(tc.tile_pool(name="wsc", bufs=4))
        w1Tp = ctx.enter_context(tc.tile_pool(name="w1T", bufs=2))
        w3Tp = ctx.enter_context(tc.tile_pool(name="w3T", bufs=2))
        w2Tp = ctx.enter_context(tc.tile_pool(name="w2T", bufs=2))
        pstg = ctx.enter_context(tc.tile_pool(name="pstg", bufs=2, space="PSUM"))
        pmm = ctx.enter_context(tc.tile_pool(name="pmm", bufs=4, space="PSUM"))
        spost = ctx.enter_context(tc.tile_pool(name="spost", bufs=2))

        def dequant_w13_T(qd, s_sb, i, wT):
            # W rows [i*128,(i+1)*128) -> wT[h-part, f-free] fp16, scaled
            halves = []
            for half in range(2):
                sl = slab.tile([128, HHALF], dt.int32, tag="slab")
                nc.sync.dma_start(
                    sl[:],
                    qd.ap()[
                        i * 128 : (i + 1) * 128,
                        half * HHALF : (half + 1) * HHALF,
                    ],
                )
                ws = wscp.tile([128, HHALF], dt.float16, tag="wsc")
                s_ap = s_sb[
                    :, i * NG1 + half * (NG1 // 2) : i * NG1 + (half + 1) * (NG1 // 2)
                ]
                nc.vector.tensor_tensor(
                    ws[:].rearrange("p (g k) -> p g k", k=G),
                    sl[:].rearrange("p (g k) -> p g k", k=G),
                    bcast(s_ap, G),
                    op=Alu.mult,
                )
                halves.append(ws)
            for q in range(8):
                stg = pstg.tile([128, 512], dt.float16, tag="stg")
                for k in range(4):
                    c = q * 4 + k
                    half, cc = divmod(c, 16)
                    nc.tensor.transpose(
                        stg[:, k * 128 : (k + 1) * 128],
                        halves[half][:, cc * 128 : (cc + 1) * 128],
                        eye_hf[:],
                    )
                nc.scalar.activation(wT[:, q * 512 : (q + 1) * 512], stg[:], Act.Copy)

        # ---- phase 1: gT = silu(x@W1.T).T * (x@W3.T).T, [f-part, t-free] ----
        for i in range(NF):
            w1T = w1Tp.tile([128, H], dt.float16, tag="w1T")
            dequant_w13_T(w1q_d, s1_sb, i, w1T)
            w3T = w3Tp.tile([128, H], dt.float16, tag="w3T")
            dequant_w13_T(w3q_d, s3_sb, i, w3T)

            g1 = pmm.tile([128, T], dt.float32, tag="mm")
            for c in range(NH):
                nc.tensor.matmul(
                    g1[:],
                    w1T[:, c * 128 : (c + 1) * 128],
                    xT_hf[:, c * T : (c + 1) * T],
                    start=(c == 0),
                    stop=False,
                )
            nc.tensor.matmul(
                g1[:], negZS1T[:, i * 128 : (i + 1) * 128], XgT[:],
                start=False, stop=True,
            )
            g3 = pmm.tile([128, T], dt.float32, tag="mm")
            for c in range(NH):
                nc.tensor.matmul(
                    g3[:],
                    w3T[:, c * 128 : (c + 1) * 128],
                    xT_hf[:, c * T : (c + 1) * T],
                    start=(c == 0),
                    stop=False,
                )
            nc.tensor.matmul(
                g3[:], negZS3T[:, i * 128 : (i + 1) * 128], XgT[:],
                start=False, stop=True,
            )
            # silu(G1)*G3 = G1*sigmoid(G1)*G3 (Sigmoid: real ACT LUT, also in sim)
            st = spost.tile([128, T], dt.float16, tag="silu")
            nc.scalar.activation(st[:], g1[:], Act.Sigmoid)
            t2 = spost.tile([128, T], dt.float16, tag="silu2")
            nc.vector.tensor_tensor(t2[:], st[:], g3[:], op=Alu.mult)
            nc.vector.tensor_tensor(
                gT_hf[:, i * T : (i + 1) * T], t2[:], g1[:], op=Alu.mult
            )

        # ---- Gg.T: per-group sums of gated activations ----
        gg_ps = pmisc.tile([GC2, T], dt.float32, tag="misc")
        for ii in range(NF):
            nc.tensor.matmul(
                gg_ps[:],
                B2[:, ii * GC2 : (ii + 1) * GC2],
                gT_hf[:, ii * T : (ii + 1) * T],
                start=(ii == 0),
                stop=(ii == NF - 1),
            )
        nc.scalar.activation(GgT[:], gg_ps[:], Act.Copy)

        # ---- phase 2: y.T partial = (g @ W2.T).T ----
        for j in range(NH):
            sl = slab.tile([128, FC], dt.int32, tag="slab")
            nc.sync.dma_start(sl[:], w2q_d.ap()[j * 128 : (j + 1) * 128, :])
            ws = wscp.tile([128, FC], dt.float16, tag="wsc")
            nc.vector.tensor_tensor(
                ws[:].rearrange("p (g k) -> p g k", k=G),
                sl[:].rearrange("p (g k) -> p g k", k=G),
                bcast(s2_sb[:, j * GC2 : (j + 1) * GC2], G),
                op=Alu.mult,
            )
            w2T = w2Tp.tile([128, FC], dt.float16, tag="w2T")
            for b in range(2):
                stg = pstg.tile([128, 7 * 128], dt.float16, tag="stg")
                for k in range(7):
                    c = b * 7 + k
                    nc.tensor.transpose(
                        stg[:, k * 128 : (k + 1) * 128],
                        ws[:, c * 128 : (c + 1) * 128],
                        eye_hf[:],
                    )
                nc.scalar.activation(
                    w2T[:, b * 896 : (b + 1) * 896], stg[:], Act.Copy
                )
            yp = pmm.tile([128, T], dt.float32, tag="mm")
            for ii in range(NF):
                nc.tensor.matmul(
                    yp[:],
                    w2T[:, ii * 128 : (ii + 1) * 128],
                    gT_hf[:, ii * T : (ii + 1) * T],
                    start=(ii == 0),
                    stop=False,
                )
            nc.tensor.matmul(
                yp[:], negZS2T[:, j * 128 : (j + 1) * 128], GgT[:],
                start=False, stop=True,
            )
            ysb = spost.tile([128, T], dt.float32, tag="ysb")
            nc.scalar.activation(ysb[:], yp[:], Act.Copy)
            nc.sync.dma_start(yT_d.ap()[j * 128 : (j + 1) * 128, :], ysb[:])

    nc.compile()
    return nc


def _host_consts():
    eye_hf = np.eye(128, dtype=np.float16)
    eye_f32 = np.eye(128, dtype=np.float32)
    p = np.arange(128)
    B1 = np.zeros((128, NH * G), np.float16)
    for c in range(NH):
        B1[p, c * G + 2 * c + p // 64] = 1.0
    B2 = np.zeros((128, NF * GC2), np.float16)
    for ii in range(NF):
        B2[p, ii * GC2 + 2 * ii + p // 64] = 1.0
    return eye_hf, eye_f32, B1, B2


def _in_maps(inputs):
    x = np.asarray(inputs["hidden_states"], dtype=np.float32)
    xT = np.ascontiguousarray(x.T)
    W1 = np.asarray(inputs["W1_q"], dtype=np.int32)
    W3 = np.asarray(inputs["W3_q"], dtype=np.int32)
    W2 = np.asarray(inputs["W2_q"], dtype=np.int32)
    s1 = np.asarray(inputs["w1_scale"], dtype=np.float32)
    z1 = np.asarray(inputs["w1_zero"], dtype=np.float32)
    s3 = np.asarray(inputs["w3_scale"], dtype=np.float32)
    z3 = np.asarray(inputs["w3_zero"], dtype=np.float32)
    s2 = np.asarray(inputs["w2_scale"], dtype=np.float32)
    z2 = np.asarray(inputs["w2_zero"], dtype=np.float32)
    eye_hf, eye_f32, B1, B2 = _host_consts()

    maps = []
    for c in range(N_CORES):
        f0 = c * FC
        g0 = c * GC2
        maps.append(
            {
                "w1q": np.ascontiguousarray(W1[f0 : f0 + FC]),
                "w3q": np.ascontiguousarray(W3[f0 : f0 + FC]),
                "w2q": np.ascontiguousarray(W2[:, f0 : f0 + FC]),
                "s1": np.ascontiguousarray(s1[f0 : f0 + FC]),
                "z1": np.ascontiguousarray(z1[f0 : f0 + FC]),
                "s3": np.ascontiguousarray(s3[f0 : f0 + FC]),
                "z3": np.ascontiguousarray(z3[f0 : f0 + FC]),
                "s2": np.ascontiguousarray(s2[:, g0 : g0 + GC2]),
                "z2": np.ascontiguousarray(z2[:, g0 : g0 + GC2]),
                "xT": xT,
                "eye_hf": eye_hf,
                "eye_f32": eye_f32,
                "B1": B1,
                "B2": B2,
            }
        )
    return maps


def _install_trace_shim():
    import sys
    import types

    if "antenv.axon_hooks" in sys.modules:
        return
    holder = {}
    mod = types.ModuleType("antenv.axon_hooks")
    mod.set_axon_ntff_profile_hook = lambda h: holder.__setitem__("h", h)
    mod.get_axon_ntff_profile_hook = lambda: holder.get("h")
    sys.modules["antenv.axon_hooks"] = mod
    try:
        sys.path.insert(0, "/root/.axon_site")
        from trn_agent_boot.trn_boot import _ntff_profile_via_ctypes

        holder["h"] = _ntff_profile_via_ctypes("/opt/axon/libaxon_pjrt.so")
    except Exception:
        holder["h"] = None


def kernel(**inputs):
    global _built
    if _built is None:
        _built = _build()
    nc = _built
    from concourse.bass_utils import run_bass_kernel_spmd

    maps = _in_maps(inputs)
    trace_dir = os.environ.get("BASSK_TRACE_DIR")
    if trace_dir:
        _install_trace_shim()
        res = run_bass_kernel_spmd(
            nc, maps, list(range(N_CORES)), trace=True, tmpdir=trace_dir
        )
    else:
        res = run_bass_kernel_spmd(nc, maps, list(range(N_CORES)))
    kernel.last_results = res

    yT = res.results[0]["yT"].astype(np.float32)
    for c in range(1, N_CORES):
        yT = yT + res.results[c]["yT"]
    return np.ascontiguousarray(yT.T)


# revision 13
# speedup vs baseline: 1.0224x; 1.0224x over previous
# Trainium2 Bass kernel for Mixtral block-sparse top-2 MLP with HQQ 4-bit
# quantized weights (dequant + silu-gated MLP), tensor-parallel over 8
# NeuronCores.
#
#   y = (silu(x @ W1.T) * (x @ W3.T)) @ W2.T
#   W* = (Wq - zero) * scale, groups of 64 along the input dim.
#
# Sharding: F (=14336) split 8 ways. Core c holds rows [c*1792, (c+1)*1792)
# of W1/W3 and the matching columns of W2, computes a partial y.T, and the
# host sums the 8 partials (the "all-reduce" of row-parallel W2).
#
# Device-side math (per core, everything fp16 except PSUM f32 accum):
#   Wsc = Wq * scale         (one broadcast-AP tensor_tensor per slab;
#                             int32 -> fp16, scale broadcast over each
#                             64-wide group)
#   zero terms are folded into a rank-|groups| correction matmul:
#     x @ ((Wq - z)*s).T = x @ (Wq*s).T - Xg @ (z*s).T
#   where Xg[t,g] = sum of x[t, h] over group g. Same trick for W2 with
#   Gg[t,fg] group sums of the gated activations.
#
# Layouts: weights are DMA'd in natural layout (dequant needs the non-group
# index on partitions), transposed on the PE (128x128 tiles via identity
# matmul) into contraction-on-partitions layout for the main matmuls.

import os
import numpy as np

N_CORES = 8
T, H, F = 512, 4096, 14336
FC = F // N_CORES        # 1792 rows of W1/W3 (cols of W2) per core
NF = FC // 128           # 14 f-tiles per core
NH = H // 128            # 32 h-tiles
G = 64                   # quantization group size
NG1 = H // G             # 64 groups along H (W1/W3)
GC2 = FC // G            # 28 groups along F per core (W2)
HHALF = H // 2           # dequant slab width for W1/W3

_built = None


def _build():
    from contextlib import ExitStack
    import concourse.bacc as bacc
    import concourse.bass as bass
    import concourse.tile as tile
    import concourse.mybir as mybir

    dt = mybir.dt
    Alu = mybir.AluOpType
    Act = mybir.ActivationFunctionType

    nc = bacc.Bacc("TRN2", target_bir_lowering=False, debug=False)

    w1q_d = nc.dram_tensor("w1q", [FC, H], dt.int32, kind="ExternalInput")
    w3q_d = nc.dram_tensor("w3q", [FC, H], dt.int32, kind="ExternalInput")
    w2q_d = nc.dram_tensor("w2q", [H, FC], dt.int32, kind="ExternalInput")
    s1_d = nc.dram_tensor("s1", [FC, NG1], dt.float32, kind="ExternalInput")
    z1_d = nc.dram_tensor("z1", [FC, NG1], dt.float32, kind="ExternalInput")
    s3_d = nc.dram_tensor("s3", [FC, NG1], dt.float32, kind="ExternalInput")
    z3_d = nc.dram_tensor("z3", [FC, NG1], dt.float32, kind="ExternalInput")
    s2_d = nc.dram_tensor("s2", [H, GC2], dt.float32, kind="ExternalInput")
    z2_d = nc.dram_tensor("z2", [H, GC2], dt.float32, kind="ExternalInput")
    xT_d = nc.dram_tensor("xT", [H, T], dt.float32, kind="ExternalInput")
    eyeh_d = nc.dram_tensor("eye_hf", [128, 128], dt.float16, kind="ExternalInput")
    eyef_d = nc.dram_tensor("eye_f32", [128, 128], dt.float32, kind="ExternalInput")
    # Bs[p, m] = 1 iff m == 64 + p//64; slice [64-2c, 128-2c) gives the
    # block-ones lhsT that sums a 128-partition chunk into group rows
    # (2c + p//64) of a group-sum matmul.
    Bs_d = nc.dram_tensor("Bs", [128, 128], dt.float16, kind="ExternalInput")
    yT_d = nc.dram_tensor("yT", [H, T], dt.float16, kind="ExternalOutput")

    def bcast(ap, n):
        # append a stride-0 dim of size n to an AP (free-dim broadcast)
        return bass.AP(ap.tensor, ap.offset, list(ap.ap) + [[0, n]])

    with tile.TileContext(nc) as tc, ExitStack() as ctx:
        cpool = ctx.enter_context(tc.tile_pool(name="consts", bufs=1))
        pmisc = ctx.enter_context(tc.tile_pool(name="pmisc", bufs=1, space="PSUM"))

        eye_hf = cpool.tile([128, 128], dt.float16, tag="eyeh")
        nc.sync.dma_start(eye_hf[:], eyeh_d.ap())
        eye_f32 = cpool.tile([128, 128], dt.float32, tag="eyef")
        nc.sync.dma_start(eye_f32[:], eyef_d.ap())
        Bs = cpool.tile([128, 128], dt.float16, tag="Bs")
        nc.sync.dma_start(Bs[:], Bs_d.ap())

        # scales in "tiled-natural" layout: s1_sb[p, i*NGpt + g] = s1[i*128+p, g]
        def load_sc(dram, ngr, ntile):
            t = cpool.tile([128, ntile * ngr], dt.float32, tag=dram.name + "sb")
            nc.sync.dma_start(
                t[:].rearrange("p (i g) -> p i g", g=ngr),
                dram.ap().rearrange("(i p) g -> p i g", p=128),
            )
            return t

        s1_sb = load_sc(s1_d, NG1, NF)
        s3_sb = load_sc(s3_d, NG1, NF)
        s2_sb = load_sc(s2_d, GC2, NH)

        negZS1T = cpool.tile([G, NF * 128], dt.float16, tag="nzs1T")
        negZS3T = cpool.tile([G, NF * 128], dt.float16, tag="nzs3T")
        negZS2T = cpool.tile([GC2, NH * 128], dt.float16, tag="nzs2T")
        XgT = cpool.tile([NG1, T], dt.float16, tag="XgT")
        GgT = cpool.tile([GC2, T], dt.float16, tag="GgT")
        xT_hf = cpool.tile([128, NH * T], dt.float16, tag="xThf")
        gT_hf = cpool.tile([128, NF * T], dt.float16, tag="gThf")

        # ---- x: load f32 in 4 chunks, cast to fp16 (scoped staging) ----
        NQ = NH // 4  # 8 h-tiles per load chunk
        with tc.tile_pool(name="xstg", bufs=1) as xstg:
            xs = xstg.tile([128, NH * T], dt.float32, tag="xs")
            for q in range(4):
                nc.sync.dma_start(
                    xs[:, q * NQ * T : (q + 1) * NQ * T].rearrange(
                        "p (i t) -> p i t", t=T
                    ),
                    xT_d.ap()[q * NQ * 128 : (q + 1) * NQ * 128, :].rearrange(
                        "(i p) t -> p i t", p=128
                    ),
                )
                nc.vector.tensor_copy(
                    xT_hf[:, q * NQ * T : (q + 1) * NQ * T],
                    xs[:, q * NQ * T : (q + 1) * NQ * T],
                )

            # ---- negated zero*scale, transposed (small) ----
            def neg_zs_T(z_dram, s_sb, ngr, ntile, outT):
                zs = xstg.tile([128, ntile * ngr], dt.float32, tag="zstg")
                nc.sync.dma_start(
                    zs[:].rearrange("p (i g) -> p i g", g=ngr),
                    z_dram.ap().rearrange("(i p) g -> p i g", p=128),
                )
                nzs = xstg.tile([128, ntile * ngr], dt.float32, tag="nzstg")
                nc.vector.scalar_tensor_tensor(
                    nzs[:], zs[:], -1.0, s_sb[:], op0=Alu.mult, op1=Alu.mult
                )
                for i in range(ntile):
                    pt = pmisc.tile([ngr, 128], dt.float32, tag="misc")
                    nc.tensor.transpose(
                        pt[:], nzs[:, i * ngr : (i + 1) * ngr], eye_f32[:]
                    )
                    nc.scalar.activation(
                        outT[:, i * 128 : (i + 1) * 128], pt[:], Act.Copy
                    )

            neg_zs_T(z1_d, s1_sb, NG1, NF, negZS1T)
            neg_zs_T(z3_d, s3_sb, NG1, NF, negZS3T)
            neg_zs_T(z2_d, s2_sb, GC2, NH, negZS2T)

        # ---- Xg.T: per-group sums of x via block-ones matmuls ----
        xg_ps = pmisc.tile([NG1, T], dt.float32, tag="misc")
        for c in range(NH):
            nc.tensor.matmul(
                xg_ps[:],
                Bs[:, 64 - 2 * c : 128 - 2 * c],
                xT_hf[:, c * T : (c + 1) * T],
                start=(c == 0),
                stop=(c == NH - 1),
            )
        nc.scalar.activation(XgT[:], xg_ps[:], Act.Copy)

        # ---- main pools ----
        slab = ctx.enter_context(tc.tile_pool(name="slab", bufs=4))
        wscp = ctx.enter_context(tc.tile_pool(name="wsc", bufs=5))
        w1Tp = ctx.enter_context(tc.tile_pool(name="w1T", bufs=2))
        w3Tp = ctx.enter_context(tc.tile_pool(name="w3T", bufs=2))
        w2Tp = ctx.enter_context(tc.tile_pool(name="w2T", bufs=4))
        pstg = ctx.enter_context(tc.tile_pool(name="pstg", bufs=3, space="PSUM"))
        pmm = ctx.enter_context(tc.tile_pool(name="pmm", bufs=4, space="PSUM"))
        spost = ctx.enter_context(tc.tile_pool(name="spost", bufs=2))

        def dequant_w13_T(qd, s_sb, i, wT):
            # W rows [i*128,(i+1)*128) -> wT[h-part, f-free] fp16, scaled
            halves = []
            for half in range(2):
                sl = slab.tile([128, HHALF], dt.int32, tag="slab")
                nc.sync.dma_start(
                    sl[:],
                    qd.ap()[
                        i * 128 : (i + 1) * 128,
                        half * HHALF : (half + 1) * HHALF,
                    ],
                )
                ws = wscp.tile([128, HHALF], dt.float16, tag="wsc")
                s_ap = s_sb[
                    :, i * NG1 + half * (NG1 // 2) : i * NG1 + (half + 1) * (NG1 // 2)
                ]
                nc.vector.tensor_tensor(
                    ws[:].rearrange("p (g k) -> p g k", k=G),
                    sl[:].rearrange("p (g k) -> p g k", k=G),
                    bcast(s_ap, G),
                    op=Alu.mult,
                )
                halves.append(ws)
            for q in range(8):
                stg = pstg.tile([128, 512], dt.float16, tag="stg")
                for k in range(4):
                    c = q * 4 + k
                    half, cc = divmod(c, 16)
                    nc.tensor.transpose(
                        stg[:, k * 128 : (k + 1) * 128],
                        halves[half][:, cc * 128 : (cc + 1) * 128],
                        eye_hf[:],
                    )
                nc.scalar.activation(wT[:, q * 512 : (q + 1) * 512], stg[:], Act.Copy)

        # W2 prep unit: DMA + dequant + transpose rows [j*128,(j+1)*128) of
        # this core's W2 shard into [f-part, h-free] fp16 tiles.
        w2T_tiles = {}

        def w2_prep(j):
            sl = slab.tile([128, FC], dt.int32, tag="slab")
            nc.sync.dma_start(sl[:], w2q_d.ap()[j * 128 : (j + 1) * 128, :])
            ws = wscp.tile([128, FC], dt.float16, tag="wsc")
            nc.vector.tensor_tensor(
                ws[:].rearrange("p (g k) -> p g k", k=G),
                sl[:].rearrange("p (g k) -> p g k", k=G),
                bcast(s2_sb[:, j * GC2 : (j + 1) * GC2], G),
                op=Alu.mult,
            )
            w2T = w2Tp.tile([128, FC], dt.float16, tag="w2T")
            for b in range(2):
                stg = pstg.tile([128, 7 * 128], dt.float16, tag="stg")
                for k in range(7):
                    c = b * 7 + k
                    nc.tensor.transpose(
                        stg[:, k * 128 : (k + 1) * 128],
                        ws[:, c * 128 : (c + 1) * 128],
                        eye_hf[:],
                    )
                nc.scalar.activation(
                    w2T[:, b * 896 : (b + 1) * 896], stg[:], Act.Copy
                )
            w2T_tiles[j] = w2T

        W2_AHEAD = 3

        # ---- phase 1: gT = silu(x@W1.T).T * (x@W3.T).T, [f-part, t-free] ----
        for i in range(NF):
            w1T = w1Tp.tile([128, H], dt.float16, tag="w1T")
            dequant_w13_T(w1q_d, s1_sb, i, w1T)
            w3T = w3Tp.tile([128, H], dt.float16, tag="w3T")
            dequant_w13_T(w3q_d, s3_sb, i, w3T)

            g1 = pmm.tile([128, T], dt.float32, tag="mm")
            for c in range(NH):
                nc.tensor.matmul(
                    g1[:],
                    w1T[:, c * 128 : (c + 1) * 128],
                    xT_hf[:, c * T : (c + 1) * T],
                    start=(c == 0),
                    stop=False,
                )
            nc.tensor.matmul(
                g1[:], negZS1T[:, i * 128 : (i + 1) * 128], XgT[:],
                start=False, stop=True,
            )
            g3 = pmm.tile([128, T], dt.float32, tag="mm")
            for c in range(NH):
                nc.tensor.matmul(
                    g3[:],
                    w3T[:, c * 128 : (c + 1) * 128],
                    xT_hf[:, c * T : (c + 1) * T],
                    start=(c == 0),
                    stop=False,
                )
            nc.tensor.matmul(
                g3[:], negZS3T[:, i * 128 : (i + 1) * 128], XgT[:],
                start=False, stop=True,
            )
            # silu(G1)*G3 = G1*sigmoid(G1)*G3 (Sigmoid: real ACT LUT, also in sim)
            st = spost.tile([128, T], dt.float16, tag="silu")
            nc.scalar.activation(st[:], g1[:], Act.Sigmoid)
            t2 = spost.tile([128, T], dt.float16, tag="silu2")
            nc.vector.tensor_tensor(t2[:], st[:], g3[:], op=Alu.mult)
            nc.vector.tensor_tensor(
                gT_hf[:, i * T : (i + 1) * T], t2[:], g1[:], op=Alu.mult
            )
            # prefetch first W2 units near the end of phase 1
            if i >= NF - W2_AHEAD:
                w2_prep(i - (NF - W2_AHEAD))

        # ---- Gg.T: per-group sums of gated activations ----
        gg_ps = pmisc.tile([GC2, T], dt.float32, tag="misc")
        for ii in range(NF):
            nc.tensor.matmul(
                gg_ps[:],
                Bs[:, 64 - 2 * ii : 92 - 2 * ii],
                gT_hf[:, ii * T : (ii + 1) * T],
                start=(ii == 0),
                stop=(ii == NF - 1),
            )
        nc.scalar.activation(GgT[:], gg_ps[:], Act.Copy)

        # ---- phase 2: y.T partial = (g @ W2.T).T, y-writes batched by 4 ----
        ysb = None
        for j in range(NH):
            if j + W2_AHEAD < NH:
                w2_prep(j + W2_AHEAD)
            w2T = w2T_tiles.pop(j)
            yp = pmm.tile([128, T], dt.float32, tag="mm")
            for ii in range(NF):
                nc.tensor.matmul(
                    yp[:],
                    w2T[:, ii * 128 : (ii + 1) * 128],
                    gT_hf[:, ii * T : (ii + 1) * T],
                    start=(ii == 0),
                    stop=False,
                )
            nc.tensor.matmul(
                yp[:], negZS2T[:, j * 128 : (j + 1) * 128], GgT[:],
                start=False, stop=True,
            )
            if j % 4 == 0:
                ysb = spost.tile([128, 4 * T], dt.float16, tag="ysb")
            nc.scalar.activation(
                ysb[:, (j % 4) * T : (j % 4 + 1) * T], yp[:], Act.Copy
            )
            if j % 4 == 3:
                nc.sync.dma_start(
                    yT_d.ap()[(j - 3) * 128 : (j + 1) * 128, :].rearrange(
                        "(q p) t -> p q t", p=128
                    ),
                    ysb[:].rearrange("p (q t) -> p q t", t=T),
                )

    nc.compile()
    return nc


def _host_consts():
    eye_hf = np.eye(128, dtype=np.float16)
    eye_f32 = np.eye(128, dtype=np.float32)
    p = np.arange(128)
    Bs = np.zeros((128, 128), np.float16)
    Bs[p, 64 + p // 64] = 1.0
    return eye_hf, eye_f32, Bs


def _in_maps(inputs):
    x = np.asarray(inputs["hidden_states"], dtype=np.float32)
    xT = np.ascontiguousarray(x.T)
    W1 = np.asarray(inputs["W1_q"], dtype=np.int32)
    W3 = np.asarray(inputs["W3_q"], dtype=np.int32)
    W2 = np.asarray(inputs["W2_q"], dtype=np.int32)
    s1 = np.asarray(inputs["w1_scale"], dtype=np.float32)
    z1 = np.asarray(inputs["w1_zero"], dtype=np.float32)
    s3 = np.asarray(inputs["w3_scale"], dtype=np.float32)
    z3 = np.asarray(inputs["w3_zero"], dtype=np.float32)
    s2 = np.asarray(inputs["w2_scale"], dtype=np.float32)
    z2 = np.asarray(inputs["w2_zero"], dtype=np.float32)
    eye_hf, eye_f32, Bs = _host_consts()

    maps = []
    for c in range(N_CORES):
        f0 = c * FC
        g0 = c * GC2
        maps.append(
            {
                "w1q": np.ascontiguousarray(W1[f0 : f0 + FC]),
                "w3q": np.ascontiguousarray(W3[f0 : f0 + FC]),
                "w2q": np.ascontiguousarray(W2[:, f0 : f0 + FC]),
                "s1": np.ascontiguousarray(s1[f0 : f0 + FC]),
                "z1": np.ascontiguousarray(z1[f0 : f0 + FC]),
                "s3": np.ascontiguousarray(s3[f0 : f0 + FC]),
                "z3": np.ascontiguousarray(z3[f0 : f0 + FC]),
                "s2": np.ascontiguousarray(s2[:, g0 : g0 + GC2]),
                "z2": np.ascontiguousarray(z2[:, g0 : g0 + GC2]),
                "xT": xT,
                "eye_hf": eye_hf,
                "eye_f32": eye_f32,
                "Bs": Bs,
            }
        )
    return maps


def _install_trace_shim():
    import sys
    import types

    if "antenv.axon_hooks" in sys.modules:
        return
    holder = {}
    mod = types.ModuleType("antenv.axon_hooks")
    mod.set_axon_ntff_profile_hook = lambda h: holder.__setitem__("h", h)
    mod.get_axon_ntff_profile_hook = lambda: holder.get("h")
    sys.modules["antenv.axon_hooks"] = mod
    try:
        sys.path.insert(0, "/root/.axon_site")
        from trn_agent_boot.trn_boot import _ntff_profile_via_ctypes

        holder["h"] = _ntff_profile_via_ctypes("/opt/axon/libaxon_pjrt.so")
    except Exception:
        holder["h"] = None


def kernel(**inputs):
    global _built
    if _built is None:
        _built = _build()
    nc = _built
    from concourse.bass_utils import run_bass_kernel_spmd

    maps = _in_maps(inputs)
    trace_dir = os.environ.get("BASSK_TRACE_DIR")
    if trace_dir:
        _install_trace_shim()
        res = run_bass_kernel_spmd(
            nc, maps, list(range(N_CORES)), trace=True, tmpdir=trace_dir
        )
    else:
        res = run_bass_kernel_spmd(nc, maps, list(range(N_CORES)))
    kernel.last_results = res

    yT = res.results[0]["yT"].astype(np.float32)
    for c in range(1, N_CORES):
        yT += res.results[c]["yT"].astype(np.float32)
    return np.ascontiguousarray(yT.T)


# revision 15
# speedup vs baseline: 1.0378x; 1.0151x over previous
# Trainium2 Bass kernel for Mixtral block-sparse top-2 MLP with HQQ 4-bit
# quantized weights (dequant + silu-gated MLP), tensor-parallel over 8
# NeuronCores.
#
#   y = (silu(x @ W1.T) * (x @ W3.T)) @ W2.T
#   W* = (Wq - zero) * scale, groups of 64 along the input dim.
#
# Sharding: F (=14336) split 8 ways. Core c holds rows [c*1792, (c+1)*1792)
# of W1/W3 and the matching columns of W2, computes a partial y.T, and the
# host sums the 8 partials (the "all-reduce" of row-parallel W2).
#
# Device-side math (per core, everything fp16 except PSUM f32 accum):
#   Wsc = Wq * scale         (one broadcast-AP tensor_tensor per slab;
#                             int32 -> fp16, scale broadcast over each
#                             64-wide group)
#   zero terms are folded into a rank-|groups| correction matmul:
#     x @ ((Wq - z)*s).T = x @ (Wq*s).T - Xg @ (z*s).T
#   where Xg[t,g] = sum of x[t, h] over group g. Same trick for W2 with
#   Gg[t,fg] group sums of the gated activations.
#
# Layouts: weights are DMA'd in natural layout (dequant needs the non-group
# index on partitions), transposed on the PE (128x128 tiles via identity
# matmul) into contraction-on-partitions layout for the main matmuls.

import os
import numpy as np

N_CORES = 8
T, H, F = 512, 4096, 14336
FC = F // N_CORES        # 1792 rows of W1/W3 (cols of W2) per core
NF = FC // 128           # 14 f-tiles per core
NH = H // 128            # 32 h-tiles
G = 64                   # quantization group size
NG1 = H // G             # 64 groups along H (W1/W3)
GC2 = FC // G            # 28 groups along F per core (W2)
HHALF = H // 2           # dequant slab width for W1/W3

_built = None


def _build():
    from contextlib import ExitStack
    import concourse.bacc as bacc
    import concourse.bass as bass
    import concourse.tile as tile
    import concourse.mybir as mybir

    dt = mybir.dt
    Alu = mybir.AluOpType
    Act = mybir.ActivationFunctionType

    nc = bacc.Bacc("TRN2", target_bir_lowering=False, debug=False)

    w1q_d = nc.dram_tensor("w1q", [FC, H], dt.int32, kind="ExternalInput")
    w3q_d = nc.dram_tensor("w3q", [FC, H], dt.int32, kind="ExternalInput")
    w2q_d = nc.dram_tensor("w2q", [H, FC], dt.int32, kind="ExternalInput")
    s1_d = nc.dram_tensor("s1", [FC, NG1], dt.float32, kind="ExternalInput")
    z1_d = nc.dram_tensor("z1", [FC, NG1], dt.float32, kind="ExternalInput")
    s3_d = nc.dram_tensor("s3", [FC, NG1], dt.float32, kind="ExternalInput")
    z3_d = nc.dram_tensor("z3", [FC, NG1], dt.float32, kind="ExternalInput")
    s2_d = nc.dram_tensor("s2", [H, GC2], dt.float32, kind="ExternalInput")
    z2_d = nc.dram_tensor("z2", [H, GC2], dt.float32, kind="ExternalInput")
    xT_d = nc.dram_tensor("xT", [H, T], dt.float32, kind="ExternalInput")
    eyeh_d = nc.dram_tensor("eye_hf", [128, 128], dt.float16, kind="ExternalInput")
    eyef_d = nc.dram_tensor("eye_f32", [128, 128], dt.float32, kind="ExternalInput")
    # Bs[p, m] = 1 iff m == 64 + p//64; slice [64-2c, 128-2c) gives the
    # block-ones lhsT that sums a 128-partition chunk into group rows
    # (2c + p//64) of a group-sum matmul.
    Bs_d = nc.dram_tensor("Bs", [128, 128], dt.float16, kind="ExternalInput")
    yT_d = nc.dram_tensor("yT", [H, T], dt.float16, kind="ExternalOutput")

    def bcast(ap, n):
        # append a stride-0 dim of size n to an AP (free-dim broadcast)
        return bass.AP(ap.tensor, ap.offset, list(ap.ap) + [[0, n]])

    with tile.TileContext(nc) as tc, ExitStack() as ctx:
        cpool = ctx.enter_context(tc.tile_pool(name="consts", bufs=1))
        pmisc = ctx.enter_context(tc.tile_pool(name="pmisc", bufs=1, space="PSUM"))

        # Small/strided loads go on the ACT HWDGE ring (nc.scalar), weight
        # slabs on the SP ring (nc.sync), x on SWDGE (nc.gpsimd, with inline
        # f32->fp16 cast) — three parallel DMA paths so the slow strided
        # scale loads don't head-of-line-block the weight stream.
        eye_hf = cpool.tile([128, 128], dt.float16, tag="eyeh")
        nc.scalar.dma_start(eye_hf[:], eyeh_d.ap())
        Bs = cpool.tile([128, 128], dt.float16, tag="Bs")
        nc.scalar.dma_start(Bs[:], Bs_d.ap())

        # scales in "tiled-natural" layout: s1_sb[p, i*NGpt + g] = s1[i*128+p, g]
        def load_sc(dram, ngr, ntile):
            t = cpool.tile([128, ntile * ngr], dt.float32, tag=dram.name + "sb")
            nc.scalar.dma_start(
                t[:].rearrange("p (i g) -> p i g", g=ngr),
                dram.ap().rearrange("(i p) g -> p i g", p=128),
            )
            return t

        s1_sb = load_sc(s1_d, NG1, NF)
        s3_sb = load_sc(s3_d, NG1, NF)
        eye_f32 = cpool.tile([128, 128], dt.float32, tag="eyef")
        nc.scalar.dma_start(eye_f32[:], eyef_d.ap())
        s2_sb = load_sc(s2_d, GC2, NH)

        negZS1T = cpool.tile([G, NF * 128], dt.float16, tag="nzs1T")
        negZS3T = cpool.tile([G, NF * 128], dt.float16, tag="nzs3T")
        negZS2T = cpool.tile([GC2, NH * 128], dt.float16, tag="nzs2T")
        XgT = cpool.tile([NG1, T], dt.float16, tag="XgT")
        GgT = cpool.tile([GC2, T], dt.float16, tag="GgT")
        xT_hf = cpool.tile([128, NH * T], dt.float16, tag="xThf")
        gT_hf = cpool.tile([128, NF * T], dt.float16, tag="gThf")

        # ---- x: SWDGE cast-DMA straight to fp16, in 4 chunks ----
        NQ = NH // 4  # 8 h-tiles per load chunk
        for q in range(4):
            nc.gpsimd.dma_start(
                xT_hf[:, q * NQ * T : (q + 1) * NQ * T].rearrange(
                    "p (i t) -> p i t", t=T
                ),
                xT_d.ap()[q * NQ * 128 : (q + 1) * NQ * 128, :].rearrange(
                    "(i p) t -> p i t", p=128
                ),
            )

        with tc.tile_pool(name="xstg", bufs=1) as xstg:
            # ---- negated zero*scale, transposed (small) ----
            def neg_zs_T(z_dram, s_sb, ngr, ntile, outT):
                zs = xstg.tile([128, ntile * ngr], dt.float32, tag="zstg")
                nc.scalar.dma_start(
                    zs[:].rearrange("p (i g) -> p i g", g=ngr),
                    z_dram.ap().rearrange("(i p) g -> p i g", p=128),
                )
                nzs = xstg.tile([128, ntile * ngr], dt.float32, tag="nzstg")
                nc.vector.scalar_tensor_tensor(
                    nzs[:], zs[:], -1.0, s_sb[:], op0=Alu.mult, op1=Alu.mult
                )
                for i in range(ntile):
                    pt = pmisc.tile([ngr, 128], dt.float32, tag="misc")
                    nc.tensor.transpose(
                        pt[:], nzs[:, i * ngr : (i + 1) * ngr], eye_f32[:]
                    )
                    nc.scalar.activation(
                        outT[:, i * 128 : (i + 1) * 128], pt[:], Act.Copy
                    )

            neg_zs_T(z1_d, s1_sb, NG1, NF, negZS1T)
            neg_zs_T(z3_d, s3_sb, NG1, NF, negZS3T)
            neg_zs_T(z2_d, s2_sb, GC2, NH, negZS2T)

        # ---- Xg.T: per-group sums of x via block-ones matmuls ----
        xg_ps = pmisc.tile([NG1, T], dt.float32, tag="misc")
        for c in range(NH):
            nc.tensor.matmul(
                xg_ps[:],
                Bs[:, 64 - 2 * c : 128 - 2 * c],
                xT_hf[:, c * T : (c + 1) * T],
                start=(c == 0),
                stop=(c == NH - 1),
            )
        nc.scalar.activation(XgT[:], xg_ps[:], Act.Copy)

        # ---- main pools ----
        slab = ctx.enter_context(tc.tile_pool(name="slab", bufs=4))
        wscp = ctx.enter_context(tc.tile_pool(name="wsc", bufs=5))
        w1Tp = ctx.enter_context(tc.tile_pool(name="w1T", bufs=2))
        w3Tp = ctx.enter_context(tc.tile_pool(name="w3T", bufs=2))
        w2Tp = ctx.enter_context(tc.tile_pool(name="w2T", bufs=4))
        pstg = ctx.enter_context(tc.tile_pool(name="pstg", bufs=3, space="PSUM"))
        pmm = ctx.enter_context(tc.tile_pool(name="pmm", bufs=4, space="PSUM"))
        spost = ctx.enter_context(tc.tile_pool(name="spost", bufs=2))

        def dequant_w13_T(qd, s_sb, i, wT):
            # W rows [i*128,(i+1)*128) -> wT[h-part, f-free] fp16, scaled
            halves = []
            for half in range(2):
                sl = slab.tile([128, HHALF], dt.int32, tag="slab")
                nc.sync.dma_start(
                    sl[:],
                    qd.ap()[
                        i * 128 : (i + 1) * 128,
                        half * HHALF : (half + 1) * HHALF,
                    ],
                )
                ws = wscp.tile([128, HHALF], dt.float16, tag="wsc")
                s_ap = s_sb[
                    :, i * NG1 + half * (NG1 // 2) : i * NG1 + (half + 1) * (NG1 // 2)
                ]
                nc.vector.tensor_tensor(
                    ws[:].rearrange("p (g k) -> p g k", k=G),
                    sl[:].rearrange("p (g k) -> p g k", k=G),
                    bcast(s_ap, G),
                    op=Alu.mult,
                )
                halves.append(ws)
            for q in range(8):
                stg = pstg.tile([128, 512], dt.float16, tag="stg")
                for k in range(4):
                    c = q * 4 + k
                    half, cc = divmod(c, 16)
                    nc.tensor.transpose(
                        stg[:, k * 128 : (k + 1) * 128],
                        halves[half][:, cc * 128 : (cc + 1) * 128],
                        eye_hf[:],
                    )
                nc.scalar.activation(wT[:, q * 512 : (q + 1) * 512], stg[:], Act.Copy)

        # W2 prep unit: DMA + dequant + transpose rows [j*128,(j+1)*128) of
        # this core's W2 shard into [f-part, h-free] fp16 tiles.
        w2T_tiles = {}

        def w2_prep(j):
            sl = slab.tile([128, FC], dt.int32, tag="slab")
            nc.sync.dma_start(sl[:], w2q_d.ap()[j * 128 : (j + 1) * 128, :])
            ws = wscp.tile([128, FC], dt.float16, tag="wsc")
            nc.vector.tensor_tensor(
                ws[:].rearrange("p (g k) -> p g k", k=G),
                sl[:].rearrange("p (g k) -> p g k", k=G),
                bcast(s2_sb[:, j * GC2 : (j + 1) * GC2], G),
                op=Alu.mult,
            )
            w2T = w2Tp.tile([128, FC], dt.float16, tag="w2T")
            for b in range(2):
                stg = pstg.tile([128, 7 * 128], dt.float16, tag="stg")
                for k in range(7):
                    c = b * 7 + k
                    nc.tensor.transpose(
                        stg[:, k * 128 : (k + 1) * 128],
                        ws[:, c * 128 : (c + 1) * 128],
                        eye_hf[:],
                    )
                nc.scalar.activation(
                    w2T[:, b * 896 : (b + 1) * 896], stg[:], Act.Copy
                )
            w2T_tiles[j] = w2T

        W2_AHEAD = 3

        # ---- phase 1: gT = silu(x@W1.T).T * (x@W3.T).T, [f-part, t-free] ----
        for i in range(NF):
            w1T = w1Tp.tile([128, H], dt.float16, tag="w1T")
            dequant_w13_T(w1q_d, s1_sb, i, w1T)
            w3T = w3Tp.tile([128, H], dt.float16, tag="w3T")
            dequant_w13_T(w3q_d, s3_sb, i, w3T)

            g1 = pmm.tile([128, T], dt.float32, tag="mm")
            for c in range(NH):
                nc.tensor.matmul(
                    g1[:],
                    w1T[:, c * 128 : (c + 1) * 128],
                    xT_hf[:, c * T : (c + 1) * T],
                    start=(c == 0),
                    stop=False,
                )
            nc.tensor.matmul(
                g1[:], negZS1T[:, i * 128 : (i + 1) * 128], XgT[:],
                start=False, stop=True,
            )
            g3 = pmm.tile([128, T], dt.float32, tag="mm")
            for c in range(NH):
                nc.tensor.matmul(
                    g3[:],
                    w3T[:, c * 128 : (c + 1) * 128],
                    xT_hf[:, c * T : (c + 1) * T],
                    start=(c == 0),
                    stop=False,
                )
            nc.tensor.matmul(
                g3[:], negZS3T[:, i * 128 : (i + 1) * 128], XgT[:],
                start=False, stop=True,
            )
            # silu(G1)*G3 = G1*sigmoid(G1)*G3 (Sigmoid: real ACT LUT, also in sim)
            st = spost.tile([128, T], dt.float16, tag="silu")
            nc.scalar.activation(st[:], g1[:], Act.Sigmoid)
            t2 = spost.tile([128, T], dt.float16, tag="silu2")
            nc.vector.tensor_tensor(t2[:], st[:], g3[:], op=Alu.mult)
            nc.vector.tensor_tensor(
                gT_hf[:, i * T : (i + 1) * T], t2[:], g1[:], op=Alu.mult
            )
            # prefetch first W2 units near the end of phase 1
            if i >= NF - W2_AHEAD:
                w2_prep(i - (NF - W2_AHEAD))

        # ---- Gg.T: per-group sums of gated activations ----
        gg_ps = pmisc.tile([GC2, T], dt.float32, tag="misc")
        for ii in range(NF):
            nc.tensor.matmul(
                gg_ps[:],
                Bs[:, 64 - 2 * ii : 92 - 2 * ii],
                gT_hf[:, ii * T : (ii + 1) * T],
                start=(ii == 0),
                stop=(ii == NF - 1),
            )
        nc.scalar.activation(GgT[:], gg_ps[:], Act.Copy)

        # ---- phase 2: y.T partial = (g @ W2.T).T, y-writes batched by 4 ----
        ysb = None
        for j in range(NH):
            if j + W2_AHEAD < NH:
                w2_prep(j + W2_AHEAD)
            w2T = w2T_tiles.pop(j)
            yp = pmm.tile([128, T], dt.float32, tag="mm")
            for ii in range(NF):
                nc.tensor.matmul(
                    yp[:],
                    w2T[:, ii * 128 : (ii + 1) * 128],
                    gT_hf[:, ii * T : (ii + 1) * T],
                    start=(ii == 0),
                    stop=False,
                )
            nc.tensor.matmul(
                yp[:], negZS2T[:, j * 128 : (j + 1) * 128], GgT[:],
                start=False, stop=True,
            )
            if j % 4 == 0:
                ysb = spost.tile([128, 4 * T], dt.float16, tag="ysb")
            nc.scalar.activation(
                ysb[:, (j % 4) * T : (j % 4 + 1) * T], yp[:], Act.Copy
            )
            if j % 4 == 3:
                nc.sync.dma_start(
                    yT_d.ap()[(j - 3) * 128 : (j + 1) * 128, :].rearrange(
                        "(q p) t -> p q t", p=128
                    ),
                    ysb[:].rearrange("p (q t) -> p q t", t=T),
                )

    nc.compile()
    return nc


def _host_consts():
    eye_hf = np.eye(128, dtype=np.float16)
    eye_f32 = np.eye(128, dtype=np.float32)
    p = np.arange(128)
    Bs = np.zeros((128, 128), np.float16)
    Bs[p, 64 + p // 64] = 1.0
    return eye_hf, eye_f32, Bs


def _in_maps(inputs):
    x = np.asarray(inputs["hidden_states"], dtype=np.float32)
    xT = np.ascontiguousarray(x.T)
    W1 = np.asarray(inputs["W1_q"], dtype=np.int32)
    W3 = np.asarray(inputs["W3_q"], dtype=np.int32)
    W2 = np.asarray(inputs["W2_q"], dtype=np.int32)
    s1 = np.asarray(inputs["w1_scale"], dtype=np.float32)
    z1 = np.asarray(inputs["w1_zero"], dtype=np.float32)
    s3 = np.asarray(inputs["w3_scale"], dtype=np.float32)
    z3 = np.asarray(inputs["w3_zero"], dtype=np.float32)
    s2 = np.asarray(inputs["w2_scale"], dtype=np.float32)
    z2 = np.asarray(inputs["w2_zero"], dtype=np.float32)
    eye_hf, eye_f32, Bs = _host_consts()

    maps = []
    for c in range(N_CORES):
        f0 = c * FC
        g0 = c * GC2
        maps.append(
            {
                "w1q": np.ascontiguousarray(W1[f0 : f0 + FC]),
                "w3q": np.ascontiguousarray(W3[f0 : f0 + FC]),
                "w2q": np.ascontiguousarray(W2[:, f0 : f0 + FC]),
                "s1": np.ascontiguousarray(s1[f0 : f0 + FC]),
                "z1": np.ascontiguousarray(z1[f0 : f0 + FC]),
                "s3": np.ascontiguousarray(s3[f0 : f0 + FC]),
                "z3": np.ascontiguousarray(z3[f0 : f0 + FC]),
                "s2": np.ascontiguousarray(s2[:, g0 : g0 + GC2]),
                "z2": np.ascontiguousarray(z2[:, g0 : g0 + GC2]),
                "xT": xT,
                "eye_hf": eye_hf,
                "eye_f32": eye_f32,
                "Bs": Bs,
            }
        )
    return maps


def _install_trace_shim():
    import sys
    import types

    if "antenv.axon_hooks" in sys.modules:
        return
    holder = {}
    mod = types.ModuleType("antenv.axon_hooks")
    mod.set_axon_ntff_profile_hook = lambda h: holder.__setitem__("h", h)
    mod.get_axon_ntff_profile_hook = lambda: holder.get("h")
    sys.modules["antenv.axon_hooks"] = mod
    try:
        sys.path.insert(0, "/root/.axon_site")
        from trn_agent_boot.trn_boot import _ntff_profile_via_ctypes

        holder["h"] = _ntff_profile_via_ctypes("/opt/axon/libaxon_pjrt.so")
    except Exception:
        holder["h"] = None


def kernel(**inputs):
    global _built
    if _built is None:
        _built = _build()
    nc = _built
    from concourse.bass_utils import run_bass_kernel_spmd

    maps = _in_maps(inputs)
    trace_dir = os.environ.get("BASSK_TRACE_DIR")
    if trace_dir:
        _install_trace_shim()
        res = run_bass_kernel_spmd(
            nc, maps, list(range(N_CORES)), trace=True, tmpdir=trace_dir
        )
    else:
        res = run_bass_kernel_spmd(nc, maps, list(range(N_CORES)))
    kernel.last_results = res

    yT = res.results[0]["yT"].astype(np.float32)
    for c in range(1, N_CORES):
        yT += res.results[c]["yT"].astype(np.float32)
    return np.ascontiguousarray(yT.T)


# revision 20
# speedup vs baseline: 1.0813x; 1.0419x over previous
# Trainium2 Bass kernel for Mixtral block-sparse top-2 MLP with HQQ 4-bit
# quantized weights (dequant + silu-gated MLP), tensor-parallel over 8
# NeuronCores.
#
#   y = (silu(x @ W1.T) * (x @ W3.T)) @ W2.T
#   W* = (Wq - zero) * scale, groups of 64 along the input dim.
#
# Sharding: F (=14336) split 8 ways. Core c holds rows [c*1792, (c+1)*1792)
# of W1/W3 and the matching columns of W2, computes a partial y.T, and the
# host sums the 8 partials (the "all-reduce" of row-parallel W2).
#
# Device-side math (per core, everything fp16 except PSUM f32 accum):
#   Wsc = Wq * scale         (one broadcast-AP tensor_tensor per slab;
#                             int32 -> fp16, scale broadcast over each
#                             64-wide group)
#   zero terms are folded into a rank-|groups| correction matmul:
#     x @ ((Wq - z)*s).T = x @ (Wq*s).T - Xg @ (z*s).T
#   where Xg[t,g] = sum of x[t, h] over group g. Same trick for W2 with
#   Gg[t,fg] group sums of the gated activations.
#
# Layouts: weights are DMA'd in natural layout (dequant needs the non-group
# index on partitions), transposed on the PE (128x128 tiles via identity
# matmul) into contraction-on-partitions layout for the main matmuls.

import os
import numpy as np

N_CORES = 8
T, H, F = 512, 4096, 14336
FC = F // N_CORES        # 1792 rows of W1/W3 (cols of W2) per core
NF = FC // 128           # 14 f-tiles per core
NH = H // 128            # 32 h-tiles
G = 64                   # quantization group size
NG1 = H // G             # 64 groups along H (W1/W3)
GC2 = FC // G            # 28 groups along F per core (W2)
HHALF = H // 2           # dequant slab width for W1/W3

_built = None


def _build():
    from contextlib import ExitStack
    import concourse.bacc as bacc
    import concourse.bass as bass
    import concourse.tile as tile
    import concourse.mybir as mybir

    dt = mybir.dt
    Alu = mybir.AluOpType
    Act = mybir.ActivationFunctionType

    nc = bacc.Bacc("TRN2", target_bir_lowering=False, debug=False)

    w1q_d = nc.dram_tensor("w1q", [FC, H], dt.int32, kind="ExternalInput")
    w3q_d = nc.dram_tensor("w3q", [FC, H], dt.int32, kind="ExternalInput")
    w2q_d = nc.dram_tensor("w2q", [H, FC], dt.int32, kind="ExternalInput")
    # scale/zero arrays come host-pre-tiled to SBUF layout:
    # s1[p, i*NG1+g] = w1_scale[i*128+p, g] etc — plain contiguous loads.
    s1_d = nc.dram_tensor("s1", [128, NF * NG1], dt.float32, kind="ExternalInput")
    z1_d = nc.dram_tensor("z1", [128, NF * NG1], dt.float32, kind="ExternalInput")
    s3_d = nc.dram_tensor("s3", [128, NF * NG1], dt.float32, kind="ExternalInput")
    z3_d = nc.dram_tensor("z3", [128, NF * NG1], dt.float32, kind="ExternalInput")
    s2_d = nc.dram_tensor("s2", [128, NH * GC2], dt.float32, kind="ExternalInput")
    z2_d = nc.dram_tensor("z2", [128, NH * GC2], dt.float32, kind="ExternalInput")
    xT_d = nc.dram_tensor("xT", [H, T], dt.float32, kind="ExternalInput")
    eyeh_d = nc.dram_tensor("eye_hf", [128, 128], dt.float16, kind="ExternalInput")
    eyef_d = nc.dram_tensor("eye_f32", [128, 128], dt.float32, kind="ExternalInput")
    # Bs[p, m] = 1 iff m == 64 + p//64; slice [64-2c, 128-2c) gives the
    # block-ones lhsT that sums a 128-partition chunk into group rows
    # (2c + p//64) of a group-sum matmul.
    Bs_d = nc.dram_tensor("Bs", [128, 128], dt.float16, kind="ExternalInput")
    yT_d = nc.dram_tensor("yT", [H, T], dt.float16, kind="ExternalOutput")

    def bcast(ap, n):
        # append a stride-0 dim of size n to an AP (free-dim broadcast)
        return bass.AP(ap.tensor, ap.offset, list(ap.ap) + [[0, n]])

    with tile.TileContext(nc) as tc, ExitStack() as ctx:
        cpool = ctx.enter_context(tc.tile_pool(name="consts", bufs=1))
        pmisc = ctx.enter_context(tc.tile_pool(name="pmisc", bufs=1, space="PSUM"))

        # Small/strided loads go on the ACT HWDGE ring (nc.scalar), weight
        # slabs on the SP ring (nc.sync), x on SWDGE (nc.gpsimd, with inline
        # f32->fp16 cast) — three parallel DMA paths so the slow strided
        # scale loads don't head-of-line-block the weight stream.
        eye_hf = cpool.tile([128, 128], dt.float16, tag="eyeh")
        nc.scalar.dma_start(eye_hf[:], eyeh_d.ap())
        Bs = cpool.tile([128, 128], dt.float16, tag="Bs")
        nc.scalar.dma_start(Bs[:], Bs_d.ap())

        def load_sc(dram, ngr, ntile):
            t = cpool.tile([128, ntile * ngr], dt.float32, tag=dram.name + "sb")
            nc.scalar.dma_start(t[:], dram.ap())
            return t

        s1_sb = load_sc(s1_d, NG1, NF)
        s3_sb = load_sc(s3_d, NG1, NF)
        eye_f32 = cpool.tile([128, 128], dt.float32, tag="eyef")
        nc.scalar.dma_start(eye_f32[:], eyef_d.ap())
        s2_sb = load_sc(s2_d, GC2, NH)

        negZS1T = cpool.tile([G, NF * 128], dt.float16, tag="nzs1T")
        negZS3T = cpool.tile([G, NF * 128], dt.float16, tag="nzs3T")
        negZS2T = cpool.tile([GC2, NH * 128], dt.float16, tag="nzs2T")
        XgT = cpool.tile([NG1, T], dt.float16, tag="XgT")
        GgT = cpool.tile([GC2, T], dt.float16, tag="GgT")
        xT_hf = cpool.tile([128, NH * T], dt.float16, tag="xThf")
        gT_hf = cpool.tile([128, NF * T], dt.float16, tag="gThf")

        # ---- x: SWDGE cast-DMA straight to fp16, in 4 chunks ----
        NQ = NH // 4  # 8 h-tiles per load chunk
        for q in range(4):
            nc.gpsimd.dma_start(
                xT_hf[:, q * NQ * T : (q + 1) * NQ * T].rearrange(
                    "p (i t) -> p i t", t=T
                ),
                xT_d.ap()[q * NQ * 128 : (q + 1) * NQ * 128, :].rearrange(
                    "(i p) t -> p i t", p=128
                ),
            )

        with tc.tile_pool(name="xstg", bufs=1) as xstg:
            # ---- negated zero*scale, transposed (small) ----
            def neg_zs_T(z_dram, s_sb, ngr, ntile, outT):
                zs = xstg.tile([128, ntile * ngr], dt.float32, tag="zstg")
                nc.scalar.dma_start(zs[:], z_dram.ap())
                nzs = xstg.tile([128, ntile * ngr], dt.float32, tag="nzstg")
                nc.vector.scalar_tensor_tensor(
                    nzs[:], zs[:], -1.0, s_sb[:], op0=Alu.mult, op1=Alu.mult
                )
                for i in range(ntile):
                    pt = pmisc.tile([ngr, 128], dt.float32, tag="misc")
                    nc.tensor.transpose(
                        pt[:], nzs[:, i * ngr : (i + 1) * ngr], eye_f32[:]
                    )
                    nc.scalar.activation(
                        outT[:, i * 128 : (i + 1) * 128], pt[:], Act.Copy
                    )

            neg_zs_T(z1_d, s1_sb, NG1, NF, negZS1T)
            neg_zs_T(z3_d, s3_sb, NG1, NF, negZS3T)
            neg_zs_T(z2_d, s2_sb, GC2, NH, negZS2T)

        # ---- Xg.T: per-group sums of x via block-ones matmuls ----
        xg_ps = pmisc.tile([NG1, T], dt.float32, tag="misc")
        for c in range(NH):
            nc.tensor.matmul(
                xg_ps[:],
                Bs[:, 64 - 2 * c : 128 - 2 * c],
                xT_hf[:, c * T : (c + 1) * T],
                start=(c == 0),
                stop=(c == NH - 1),
            )
        nc.scalar.activation(XgT[:], xg_ps[:], Act.Copy)

        # ---- main pools ----
        slab = ctx.enter_context(tc.tile_pool(name="slab", bufs=4))
        wscp = ctx.enter_context(tc.tile_pool(name="wsc", bufs=5))
        w1Tp = ctx.enter_context(tc.tile_pool(name="w1T", bufs=2))
        w3Tp = ctx.enter_context(tc.tile_pool(name="w3T", bufs=2))
        w2Tp = ctx.enter_context(tc.tile_pool(name="w2T", bufs=4))
        pstg = ctx.enter_context(tc.tile_pool(name="pstg", bufs=3, space="PSUM"))
        pmm = ctx.enter_context(tc.tile_pool(name="pmm", bufs=4, space="PSUM"))
        spost = ctx.enter_context(tc.tile_pool(name="spost", bufs=2))

        def dequant_w13_T(qd, s_sb, i, wT):
            # W rows [i*128,(i+1)*128) -> wT[h-part, f-free] fp16, scaled
            halves = []
            for half in range(2):
                sl = slab.tile([128, HHALF], dt.int32, tag="slab")
                nc.sync.dma_start(
                    sl[:],
                    qd.ap()[
                        i * 128 : (i + 1) * 128,
                        half * HHALF : (half + 1) * HHALF,
                    ],
                )
                ws = wscp.tile([128, HHALF], dt.float16, tag="wsc")
                s_ap = s_sb[
                    :, i * NG1 + half * (NG1 // 2) : i * NG1 + (half + 1) * (NG1 // 2)
                ]
                nc.vector.tensor_tensor(
                    ws[:].rearrange("p (g k) -> p g k", k=G),
                    sl[:].rearrange("p (g k) -> p g k", k=G),
                    bcast(s_ap, G),
                    op=Alu.mult,
                )
                halves.append(ws)
            for q in range(8):
                stg = pstg.tile([128, 512], dt.float16, tag="stg")
                for k in range(4):
                    c = q * 4 + k
                    half, cc = divmod(c, 16)
                    nc.tensor.transpose(
                        stg[:, k * 128 : (k + 1) * 128],
                        halves[half][:, cc * 128 : (cc + 1) * 128],
                        eye_hf[:],
                    )
                nc.scalar.activation(wT[:, q * 512 : (q + 1) * 512], stg[:], Act.Copy)

        # W2 prep unit: DMA + dequant + transpose rows [j*128,(j+1)*128) of
        # this core's W2 shard into [f-part, h-free] fp16 tiles.
        w2T_tiles = {}

        def w2_prep(j):
            sl = slab.tile([128, FC], dt.int32, tag="slab")
            nc.sync.dma_start(sl[:], w2q_d.ap()[j * 128 : (j + 1) * 128, :])
            ws = wscp.tile([128, FC], dt.float16, tag="wsc")
            nc.vector.tensor_tensor(
                ws[:].rearrange("p (g k) -> p g k", k=G),
                sl[:].rearrange("p (g k) -> p g k", k=G),
                bcast(s2_sb[:, j * GC2 : (j + 1) * GC2], G),
                op=Alu.mult,
            )
            w2T = w2Tp.tile([128, FC], dt.float16, tag="w2T")
            for b in range(2):
                stg = pstg.tile([128, 7 * 128], dt.float16, tag="stg")
                for k in range(7):
                    c = b * 7 + k
                    nc.tensor.transpose(
                        stg[:, k * 128 : (k + 1) * 128],
                        ws[:, c * 128 : (c + 1) * 128],
                        eye_hf[:],
                    )
                nc.scalar.activation(
                    w2T[:, b * 896 : (b + 1) * 896], stg[:], Act.Copy
                )
            w2T_tiles[j] = w2T

        W2_AHEAD = 3

        # ---- phase 1: gT = silu(x@W1.T).T * (x@W3.T).T, [f-part, t-free] ----
        for i in range(NF):
            w1T = w1Tp.tile([128, H], dt.float16, tag="w1T")
            dequant_w13_T(w1q_d, s1_sb, i, w1T)
            w3T = w3Tp.tile([128, H], dt.float16, tag="w3T")
            dequant_w13_T(w3q_d, s3_sb, i, w3T)

            g1 = pmm.tile([128, T], dt.float32, tag="mm")
            for c in range(NH):
                nc.tensor.matmul(
                    g1[:],
                    w1T[:, c * 128 : (c + 1) * 128],
                    xT_hf[:, c * T : (c + 1) * T],
                    start=(c == 0),
                    stop=False,
                )
            nc.tensor.matmul(
                g1[:], negZS1T[:, i * 128 : (i + 1) * 128], XgT[:],
                start=False, stop=True,
            )
            g3 = pmm.tile([128, T], dt.float32, tag="mm")
            for c in range(NH):
                nc.tensor.matmul(
                    g3[:],
                    w3T[:, c * 128 : (c + 1) * 128],
                    xT_hf[:, c * T : (c + 1) * T],
                    start=(c == 0),
                    stop=False,
                )
            nc.tensor.matmul(
                g3[:], negZS3T[:, i * 128 : (i + 1) * 128], XgT[:],
                start=False, stop=True,
            )
            # silu(G1)*G3 = G1*sigmoid(G1)*G3 (Sigmoid: real ACT LUT, also in sim)
            st = spost.tile([128, T], dt.float16, tag="silu")
            nc.scalar.activation(st[:], g1[:], Act.Sigmoid)
            t2 = spost.tile([128, T], dt.float16, tag="silu2")
            nc.vector.tensor_tensor(t2[:], st[:], g3[:], op=Alu.mult)
            nc.vector.tensor_tensor(
                gT_hf[:, i * T : (i + 1) * T], t2[:], g1[:], op=Alu.mult
            )
            # prefetch first W2 units near the end of phase 1
            if i >= NF - W2_AHEAD:
                w2_prep(i - (NF - W2_AHEAD))

        # ---- Gg.T: per-group sums of gated activations ----
        gg_ps = pmisc.tile([GC2, T], dt.float32, tag="misc")
        for ii in range(NF):
            nc.tensor.matmul(
                gg_ps[:],
                Bs[:, 64 - 2 * ii : 92 - 2 * ii],
                gT_hf[:, ii * T : (ii + 1) * T],
                start=(ii == 0),
                stop=(ii == NF - 1),
            )
        nc.scalar.activation(GgT[:], gg_ps[:], Act.Copy)

        # ---- phase 2: y.T partial = (g @ W2.T).T, y-writes batched by 4 ----
        ysb = None
        for j in range(NH):
            if j + W2_AHEAD < NH:
                w2_prep(j + W2_AHEAD)
            w2T = w2T_tiles.pop(j)
            yp = pmm.tile([128, T], dt.float32, tag="mm")
            for ii in range(NF):
                nc.tensor.matmul(
                    yp[:],
                    w2T[:, ii * 128 : (ii + 1) * 128],
                    gT_hf[:, ii * T : (ii + 1) * T],
                    start=(ii == 0),
                    stop=False,
                )
            nc.tensor.matmul(
                yp[:], negZS2T[:, j * 128 : (j + 1) * 128], GgT[:],
                start=False, stop=True,
            )
            if j % 4 == 0:
                ysb = spost.tile([128, 4 * T], dt.float16, tag="ysb")
            nc.vector.tensor_copy(ysb[:, (j % 4) * T : (j % 4 + 1) * T], yp[:])
            if j % 4 == 3:
                nc.sync.dma_start(
                    yT_d.ap()[(j - 3) * 128 : (j + 1) * 128, :].rearrange(
                        "(q p) t -> p q t", p=128
                    ),
                    ysb[:].rearrange("p (q t) -> p q t", t=T),
                )

    nc.compile()
    return nc


def _host_consts():
    eye_hf = np.eye(128, dtype=np.float16)
    eye_f32 = np.eye(128, dtype=np.float32)
    p = np.arange(128)
    Bs = np.zeros((128, 128), np.float16)
    Bs[p, 64 + p // 64] = 1.0
    return eye_hf, eye_f32, Bs


def _in_maps(inputs):
    x = np.asarray(inputs["hidden_states"], dtype=np.float32)
    xT = np.ascontiguousarray(x.T)
    W1 = np.asarray(inputs["W1_q"], dtype=np.int32)
    W3 = np.asarray(inputs["W3_q"], dtype=np.int32)
    W2 = np.asarray(inputs["W2_q"], dtype=np.int32)
    s1 = np.asarray(inputs["w1_scale"], dtype=np.float32)
    z1 = np.asarray(inputs["w1_zero"], dtype=np.float32)
    s3 = np.asarray(inputs["w3_scale"], dtype=np.float32)
    z3 = np.asarray(inputs["w3_zero"], dtype=np.float32)
    s2 = np.asarray(inputs["w2_scale"], dtype=np.float32)
    z2 = np.asarray(inputs["w2_zero"], dtype=np.float32)
    eye_hf, eye_f32, Bs = _host_consts()

    def tile13(a):  # [FC, NG1] -> [128, NF*NG1], s[p, i*NG1+g] = a[i*128+p, g]
        return np.ascontiguousarray(
            a.reshape(NF, 128, NG1).transpose(1, 0, 2).reshape(128, NF * NG1)
        )

    def tile2(a):  # [H, GC2] -> [128, NH*GC2]
        return np.ascontiguousarray(
            a.reshape(NH, 128, GC2).transpose(1, 0, 2).reshape(128, NH * GC2)
        )

    maps = []
    for c in range(N_CORES):
        f0 = c * FC
        g0 = c * GC2
        maps.append(
            {
                "w1q": np.ascontiguousarray(W1[f0 : f0 + FC]),
                "w3q": np.ascontiguousarray(W3[f0 : f0 + FC]),
                "w2q": np.ascontiguousarray(W2[:, f0 : f0 + FC]),
                "s1": tile13(s1[f0 : f0 + FC]),
                "z1": tile13(z1[f0 : f0 + FC]),
                "s3": tile13(s3[f0 : f0 + FC]),
                "z3": tile13(z3[f0 : f0 + FC]),
                "s2": tile2(s2[:, g0 : g0 + GC2]),
                "z2": tile2(z2[:, g0 : g0 + GC2]),
                "xT": xT,
                "eye_hf": eye_hf,
                "eye_f32": eye_f32,
                "Bs": Bs,
            }
        )
    return maps


def _install_trace_shim():
    import sys
    import types

    if "antenv.axon_hooks" in sys.modules:
        return
    holder = {}
    mod = types.ModuleType("antenv.axon_hooks")
    mod.set_axon_ntff_profile_hook = lambda h: holder.__setitem__("h", h)
    mod.get_axon_ntff_profile_hook = lambda: holder.get("h")
    sys.modules["antenv.axon_hooks"] = mod
    try:
        sys.path.insert(0, "/root/.axon_site")
        from trn_agent_boot.trn_boot import _ntff_profile_via_ctypes

        holder["h"] = _ntff_profile_via_ctypes("/opt/axon/libaxon_pjrt.so")
    except Exception:
        holder["h"] = None


def kernel(**inputs):
    global _built
    if _built is None:
        _built = _build()
    nc = _built
    from concourse.bass_utils import run_bass_kernel_spmd

    maps = _in_maps(inputs)
    trace_dir = os.environ.get("BASSK_TRACE_DIR")
    if trace_dir:
        _install_trace_shim()
        res = run_bass_kernel_spmd(
            nc, maps, list(range(N_CORES)), trace=True, tmpdir=trace_dir
        )
    else:
        res = run_bass_kernel_spmd(nc, maps, list(range(N_CORES)))
    kernel.last_results = res

    yT = res.results[0]["yT"].astype(np.float32)
    for c in range(1, N_CORES):
        yT += res.results[c]["yT"].astype(np.float32)
    return np.ascontiguousarray(yT.T)
